# revision 1
# baseline (speedup 1.0000x reference)
"""Bass/Trainium2 kernel for nn_BranchedPolicyNetwork.

Computes out = tanh(features @ Wr + br) where
  features: [32768, 1024] f32
  W:        [64, 2, 1024] f32  (stacked per-branch Linear(L, 2) weights)
  b:        [64, 2] f32
returning (out[..., 0], out[..., 1]) as two [32768, 64] f32 arrays.

Strategy: data-parallel over batch across 8 NeuronCores (4096 rows each).
The TensorEngine contracts over the partition dim, so features are repacked
host-side into a transposed, tile-contiguous layout (free w.r.t. HW time).

fp32 matmuls run at ~half rate on TRN2 (two HI/LO passes), which makes PE
the bottleneck (~76 us/core).  Instead we use a split-precision scheme with
fp32-level accuracy: x = xh + xl and w = wh + wl with fp16 hi/lo pairs, and
  x @ w ~= xh@wh + xl@wh + xh@wl        (xl@wl ~ 2^-22 rel, dropped)
accumulated in fp32 PSUM.  Three fp16 matmuls run ~1.7x faster than one
fp32 matmul pair, and total HBM traffic is unchanged (2x16-bit = 32-bit),
leaving the kernel at the exact-precision memory roofline (~53 us/core).
"""

import sys

for _p in ("/opt/trn_rl_repo", "/root/.axon_site"):
    if _p not in sys.path:
        sys.path.insert(0, _p)

import numpy as np

import concourse.mybir as mybir
import concourse.tile as tile
from concourse import bacc
from concourse.bass_utils import run_bass_kernel_spmd

# Problem shapes (hardcoded per contract)
B, L, A = 32768, 1024, 64
NCORES = 8
BS = B // NCORES          # 4096 batch rows per core
KO = L // 128             # 8 contraction slices
CH = 2 * A                # 128 output channels (c = k*64 + a)

F32 = mybir.dt.float32
F16 = mybir.dt.float16

# Chunk widths (batch columns per core).  1024-wide chunks minimize DMA count
# while keeping every matmul slab at N=512; with 4 chunks and bufs=4, every x
# tile has its own SBUF slot so all loads issue up front with no waits.
CHUNKS = [1024, 1024, 1024, 1024]
assert sum(CHUNKS) == BS
CN_MAX = max(CHUNKS)
MM_N = 512  # moving free dim per matmul (fp16 cap / one fp32 PSUM bank)

_NC = None


def _build_nc():
    nc = bacc.Bacc()
    # x is packed chunk-major on the host: for each chunk (cn columns), the
    # per-partition bytes are one contiguous (ko, n) block of KO*cn elements.
    xh = nc.dram_tensor("xh", [128, KO * BS], F16, kind="ExternalInput")
    xl = nc.dram_tensor("xl", [128, KO * BS], F16, kind="ExternalInput")
    wh = nc.dram_tensor("wh", [128, KO, CH], F16, kind="ExternalInput")
    wl = nc.dram_tensor("wl", [128, KO, CH], F16, kind="ExternalInput")
    bvec = nc.dram_tensor("bias", [CH, 1], F32, kind="ExternalInput")
    out = nc.dram_tensor("out", [CH, BS], F32, kind="ExternalOutput")

    with tile.TileContext(nc) as tc:
        with (
            tc.tile_pool(name="consts", bufs=1) as consts,
            tc.tile_pool(name="xhp", bufs=4) as xhp,
            tc.tile_pool(name="xlp", bufs=4) as xlp,
            tc.tile_pool(name="op", bufs=3) as op,
            tc.tile_pool(name="ps", bufs=3, space="PSUM") as ps,
            tc.tile_pool(name="warm", bufs=1, space="PSUM") as warm_ps,
        ):
            # PE warmup: ~10 dependency-free matmuls on zeroed tiles fill the
            # otherwise-idle window while the first loads stream in, so the
            # HAM clock gate is already at 8/8 (2.4 GHz) when real matmuls
            # start (saves the ~2x-slow cold ramp on the critical path).
            w_warm = consts.tile([128, CH], F16)
            nc.vector.memset(w_warm[:], 0.0)
            x_warm = consts.tile([128, MM_N], F16)
            nc.gpsimd.memset(x_warm[:], 0.0)
            pw = warm_ps.tile([CH, MM_N], F32)
            for i in range(10):
                nc.tensor.matmul(
                    pw[:], w_warm[:], x_warm[:], start=(i == 0), stop=(i == 9)
                )
            # Ring assignment: the Sync (SP) HWDGE ring is purely the x
            # stream in need-order.  The Scalar (ACT) ring loads the small
            # constants up front (before any ACTIVATE exists, so no convoy),
            # then does activations + out-stores; a store depends on its own
            # activation, so no convoy can form there either.
            wh_sb = consts.tile([128, KO, CH], F16)
            nc.scalar.dma_start(wh_sb[:], wh[:])
            wl_sb = consts.tile([128, KO, CH], F16)
            nc.scalar.dma_start(wl_sb[:], wl[:])
            b_sb = consts.tile([CH, 1], F32)
            nc.scalar.dma_start(b_sb[:], bvec[:])

            # Issue ALL x loads up front on the Sync ring: with bufs=4 and 4
            # chunks, every x tile has its own SBUF slot, so no load ever
            # waits on a tile release and the ring streams continuously at
            # HBM rate.  (Measured: one HWDGE ring saturates HBM by itself;
            # splitting the stream across rings was consistently slower.)
            xh_tiles = []
            xl_tiles = []
            n0 = 0
            for ci, cn in enumerate(CHUNKS):
                off = KO * n0
                src_h = xh[:, off : off + KO * cn].rearrange(
                    "p (ko n) -> p ko n", ko=KO
                )
                src_l = xl[:, off : off + KO * cn].rearrange(
                    "p (ko n) -> p ko n", ko=KO
                )
                xh_sb = xhp.tile([128, KO, CN_MAX], F16, tag="xh", name="xh_sb")[:, :, :cn]
                xl_sb = xlp.tile([128, KO, CN_MAX], F16, tag="xl", name="xl_sb")[:, :, :cn]
                # Split every chunk's loads into sub-DMAs (512 KB / 1 MB —
                # still line-rate sizes): Tile tracks deps per region, so
                # the chunk's first matmuls start as soon as the first ko
                # slices land instead of waiting for the whole 2 MB tile
                # (granularity stalls measured ~2-3 us per chunk boundary).
                hs = 1 if ci == 0 else 2
                for k0 in range(0, KO, hs):
                    nc.sync.dma_start(
                        xh_sb[:, k0 : k0 + hs], src_h[:, k0 : k0 + hs]
                    )
                for k0 in range(0, KO, 2):
                    nc.sync.dma_start(
                        xl_sb[:, k0 : k0 + 2], src_l[:, k0 : k0 + 2]
                    )
                xh_tiles.append(xh_sb)
                xl_tiles.append(xl_sb)
                n0 += cn

            n0 = 0
            for ci, cn in enumerate(CHUNKS):
                xh_sb = xh_tiles[ci]
                xl_sb = xl_tiles[ci]
                pt = ps.tile([CH, CN_MAX], F32, tag="pt", name="pt")[:, :cn]
                # Term-group-major, xl-dependent group last: the chunk's
                # compute starts as soon as wh/wl and its xh slice arrive,
                # and only the final 1/3 of the matmuls wait on xl.
                terms = [(wh_sb, xh_sb), (wl_sb, xh_sb), (wh_sb, xl_sb)]
                for ti, (w_sb, x_sb) in enumerate(terms):
                    for ko in range(KO):
                        for s0 in range(0, cn, MM_N):
                            s1 = min(s0 + MM_N, cn)
                            # start/stop are per PSUM slab (bank region)
                            nc.tensor.matmul(
                                pt[:, s0:s1],
                                w_sb[:, ko],
                                x_sb[:, ko, s0:s1],
                                start=(ti == 0 and ko == 0),
                                stop=(ti == len(terms) - 1 and ko == KO - 1),
                            )
                o_sb = op.tile([CH, CN_MAX], F32, tag="o", name="o_sb")[:, :cn]
                nc.scalar.activation(
                    o_sb[:],
                    pt[:],
                    mybir.ActivationFunctionType.Tanh,
                    bias=b_sb[:, 0:1],
                    scale=1.0,
                )
                # Store via the ACT engine's HWDGE ring: the store depends on
                # the activation anyway, and this keeps the Sync ring free to
                # stream xh loads.
                nc.scalar.dma_start(out[:, n0 : n0 + cn], o_sb[:])
                n0 += cn
    nc.compile()
    return nc


def _get_nc():
    global _NC
    if _NC is None:
        _NC = _build_nc()
    return _NC


def _pack_x(shard16):
    # shard16 [BS, L] -> chunk-major [128, KO*BS]: per partition p, chunk c
    # occupies a contiguous (ko, n) block.
    shT = shard16.T  # [L, BS] view
    parts = []
    n0 = 0
    for cn in CHUNKS:
        blk = (
            shT[:, n0 : n0 + cn]
            .reshape(KO, 128, cn)
            .transpose(1, 0, 2)
            .reshape(128, KO * cn)
        )
        parts.append(blk)
        n0 += cn
    return np.ascontiguousarray(np.concatenate(parts, axis=1))


def _shard_inputs(features, W, b):
    features = np.ascontiguousarray(features, dtype=np.float32)
    W = np.ascontiguousarray(W, dtype=np.float32)
    b = np.ascontiguousarray(b, dtype=np.float32)

    # Wr[l, c] with c = k*A + a; split into fp16 hi/lo, device layout [p, ko, c]
    wr = W.transpose(2, 1, 0).reshape(L, CH)
    wr_h = wr.astype(np.float16)
    wr_l = (wr - wr_h.astype(np.float32)).astype(np.float16)
    wh_dev = np.ascontiguousarray(wr_h.reshape(KO, 128, CH).transpose(1, 0, 2))
    wl_dev = np.ascontiguousarray(wr_l.reshape(KO, 128, CH).transpose(1, 0, 2))
    b_dev = np.ascontiguousarray(b.transpose(1, 0).reshape(CH, 1))

    in_maps = []
    for i in range(NCORES):
        sh = features[i * BS : (i + 1) * BS]  # [BS, L]
        sh_h = sh.astype(np.float16)
        sh_l = (sh - sh_h.astype(np.float32)).astype(np.float16)
        in_maps.append(
            {
                "xh": _pack_x(sh_h),
                "xl": _pack_x(sh_l),
                "wh": wh_dev,
                "wl": wl_dev,
                "bias": b_dev,
            }
        )
    return in_maps


def _gather(results):
    out0 = np.empty((B, A), dtype=np.float32)
    out1 = np.empty((B, A), dtype=np.float32)
    for i, r in enumerate(results):
        arr = r["out"].T  # [CH, BS] -> [BS, CH]
        out0[i * BS : (i + 1) * BS] = arr[:, :A]
        out1[i * BS : (i + 1) * BS] = arr[:, A:]
    return out0, out1


def _run(inputs, trace=False, trace_cores=None):
    nc = _get_nc()
    in_maps = _shard_inputs(inputs["features"], inputs["W"], inputs["b"])
    res = run_bass_kernel_spmd(
        nc,
        in_maps,
        core_ids=list(range(NCORES)),
        trace=trace,
        trace_cores=trace_cores,
    )
    return _gather(res.results), res


def kernel(features, W, b):
    (out0, out1), _ = _run({"features": features, "W": W, "b": b})
    return out0, out1



# revision 2
# speedup vs baseline: 1.6212x; 1.6212x over previous
"""Bass/Trainium2 kernel for nn_BranchedPolicyNetwork.

Computes out = tanh(features @ Wr + br) where
  features: [32768, 1024] f32
  W:        [64, 2, 1024] f32  (stacked per-branch Linear(L, 2) weights)
  b:        [64, 2] f32
returning (out[..., 0], out[..., 1]) as two [32768, 64] f32 arrays.

Strategy: data-parallel over batch across 8 NeuronCores (4096 rows each).
The TensorEngine contracts over the partition dim, so features are repacked
host-side into a transposed, tile-contiguous layout (free w.r.t. HW time).

The correctness gate is rel_l2 < 2e-2; a single fp16 matmul with fp32 PSUM
accumulation lands at ~3.7e-4, so x/w stream as plain fp16 (half the HBM
traffic of the exact-precision hi/lo scheme) and the tanh output is stored
as fp16 (upcast on host).  Per-core traffic: 8.39 MB x + 0.26 MB w +
1.05 MB out ~= 9.7 MB at ~358 GB/s -> ~27 us memory roofline; PE time for
the single fp16 term is ~13.7 us, fully hidden.
"""

import sys

for _p in ("/opt/trn_rl_repo", "/root/.axon_site"):
    if _p not in sys.path:
        sys.path.insert(0, _p)

import numpy as np

import concourse.mybir as mybir
import concourse.tile as tile
from concourse import bacc
from concourse.bass_utils import run_bass_kernel_spmd

# Problem shapes (hardcoded per contract)
B, L, A = 32768, 1024, 64
NCORES = 8
BS = B // NCORES          # 4096 batch rows per core
KO = L // 128             # 8 contraction slices
CH = 2 * A                # 128 output channels (c = k*64 + a)

F32 = mybir.dt.float32
F16 = mybir.dt.float16

# Chunk widths (batch columns per core).  1024-wide chunks minimize DMA count
# while keeping every matmul slab at N=512; with 4 chunks and bufs=4, every x
# tile has its own SBUF slot so all loads issue up front with no waits.
CHUNKS = [1024, 1024, 1024, 1024]
assert sum(CHUNKS) == BS
CN_MAX = max(CHUNKS)
MM_N = 512  # moving free dim per matmul (fp16 cap / one fp32 PSUM bank)

_NC = None


def _build_nc():
    nc = bacc.Bacc()
    # x is packed chunk-major on the host: for each chunk (cn columns), the
    # per-partition bytes are one contiguous (ko, n) block of KO*cn elements.
    xh = nc.dram_tensor("xh", [128, KO * BS], F16, kind="ExternalInput")
    wh = nc.dram_tensor("wh", [128, KO, CH], F16, kind="ExternalInput")
    bvec = nc.dram_tensor("bias", [CH, 1], F32, kind="ExternalInput")
    out = nc.dram_tensor("out", [CH, BS], F16, kind="ExternalOutput")

    with tile.TileContext(nc) as tc:
        with (
            tc.tile_pool(name="consts", bufs=1) as consts,
            tc.tile_pool(name="xhp", bufs=4) as xhp,
            tc.tile_pool(name="op", bufs=3) as op,
            tc.tile_pool(name="ps", bufs=3, space="PSUM") as ps,
            tc.tile_pool(name="warm", bufs=1, space="PSUM") as warm_ps,
        ):
            # PE warmup: ~10 dependency-free matmuls on zeroed tiles fill the
            # otherwise-idle window while the first loads stream in, so the
            # HAM clock gate is already at 8/8 (2.4 GHz) when real matmuls
            # start (saves the ~2x-slow cold ramp on the critical path).
            w_warm = consts.tile([128, CH], F16)
            nc.vector.memset(w_warm[:], 0.0)
            x_warm = consts.tile([128, MM_N], F16)
            nc.gpsimd.memset(x_warm[:], 0.0)
            pw = warm_ps.tile([CH, MM_N], F32)
            for i in range(10):
                nc.tensor.matmul(
                    pw[:], w_warm[:], x_warm[:], start=(i == 0), stop=(i == 9)
                )
            # Ring assignment: the Sync (SP) HWDGE ring is purely the x
            # stream in need-order.  The Scalar (ACT) ring loads the small
            # constants up front (before any ACTIVATE exists, so no convoy),
            # then does activations + out-stores; a store depends on its own
            # activation, so no convoy can form there either.
            wh_sb = consts.tile([128, KO, CH], F16)
            nc.scalar.dma_start(wh_sb[:], wh[:])
            b_sb = consts.tile([CH, 1], F32)
            nc.scalar.dma_start(b_sb[:], bvec[:])

            # Issue ALL x loads up front on the Sync ring: with bufs=4 and 4
            # chunks, every x tile has its own SBUF slot, so no load ever
            # waits on a tile release and the ring streams continuously at
            # HBM rate.  (Measured: one HWDGE ring saturates HBM by itself;
            # splitting the stream across rings was consistently slower.)
            xh_tiles = []
            n0 = 0
            for ci, cn in enumerate(CHUNKS):
                off = KO * n0
                src_h = xh[:, off : off + KO * cn].rearrange(
                    "p (ko n) -> p ko n", ko=KO
                )
                xh_sb = xhp.tile([128, KO, CN_MAX], F16, tag="xh", name="xh_sb")[:, :, :cn]
                # Split every chunk's loads into sub-DMAs (512 KB / 1 MB —
                # still line-rate sizes): Tile tracks deps per region, so
                # the chunk's first matmuls start as soon as the first ko
                # slices land instead of waiting for the whole tile
                # (granularity stalls measured ~2-3 us per chunk boundary).
                hs = 1 if ci == 0 else 2
                for k0 in range(0, KO, hs):
                    nc.sync.dma_start(
                        xh_sb[:, k0 : k0 + hs], src_h[:, k0 : k0 + hs]
                    )
                xh_tiles.append(xh_sb)
                n0 += cn

            n0 = 0
            for ci, cn in enumerate(CHUNKS):
                xh_sb = xh_tiles[ci]
                pt = ps.tile([CH, CN_MAX], F32, tag="pt", name="pt")[:, :cn]
                for ko in range(KO):
                    for s0 in range(0, cn, MM_N):
                        s1 = min(s0 + MM_N, cn)
                        # start/stop are per PSUM slab (bank region)
                        nc.tensor.matmul(
                            pt[:, s0:s1],
                            wh_sb[:, ko],
                            xh_sb[:, ko, s0:s1],
                            start=(ko == 0),
                            stop=(ko == KO - 1),
                        )
                o_sb = op.tile([CH, CN_MAX], F16, tag="o", name="o_sb")[:, :cn]
                nc.scalar.activation(
                    o_sb[:],
                    pt[:],
                    mybir.ActivationFunctionType.Tanh,
                    bias=b_sb[:, 0:1],
                    scale=1.0,
                )
                # Store via the ACT engine's HWDGE ring: the store depends on
                # the activation anyway, and this keeps the Sync ring free to
                # stream xh loads.
                nc.scalar.dma_start(out[:, n0 : n0 + cn], o_sb[:])
                n0 += cn
    nc.compile()
    return nc


def _get_nc():
    global _NC
    if _NC is None:
        _NC = _build_nc()
    return _NC


def _pack_x(shard16):
    # shard16 [BS, L] -> chunk-major [128, KO*BS]: per partition p, chunk c
    # occupies a contiguous (ko, n) block.
    shT = shard16.T  # [L, BS] view
    parts = []
    n0 = 0
    for cn in CHUNKS:
        blk = (
            shT[:, n0 : n0 + cn]
            .reshape(KO, 128, cn)
            .transpose(1, 0, 2)
            .reshape(128, KO * cn)
        )
        parts.append(blk)
        n0 += cn
    return np.ascontiguousarray(np.concatenate(parts, axis=1))


def _shard_inputs(features, W, b):
    features = np.ascontiguousarray(features, dtype=np.float32)
    W = np.ascontiguousarray(W, dtype=np.float32)
    b = np.ascontiguousarray(b, dtype=np.float32)

    # Wr[l, c] with c = k*A + a; fp16, device layout [p, ko, c]
    wr = W.transpose(2, 1, 0).reshape(L, CH)
    wr_h = wr.astype(np.float16)
    wh_dev = np.ascontiguousarray(wr_h.reshape(KO, 128, CH).transpose(1, 0, 2))
    b_dev = np.ascontiguousarray(b.transpose(1, 0).reshape(CH, 1))

    in_maps = []
    for i in range(NCORES):
        sh = features[i * BS : (i + 1) * BS]  # [BS, L]
        sh_h = sh.astype(np.float16)
        in_maps.append(
            {
                "xh": _pack_x(sh_h),
                "wh": wh_dev,
                "bias": b_dev,
            }
        )
    return in_maps


def _gather(results):
    out0 = np.empty((B, A), dtype=np.float32)
    out1 = np.empty((B, A), dtype=np.float32)
    for i, r in enumerate(results):
        arr = r["out"].T.astype(np.float32)  # [CH, BS] -> [BS, CH]
        out0[i * BS : (i + 1) * BS] = arr[:, :A]
        out1[i * BS : (i + 1) * BS] = arr[:, A:]
    return out0, out1


def _run(inputs, trace=False, trace_cores=None):
    nc = _get_nc()
    in_maps = _shard_inputs(inputs["features"], inputs["W"], inputs["b"])
    res = run_bass_kernel_spmd(
        nc,
        in_maps,
        core_ids=list(range(NCORES)),
        trace=trace,
        trace_cores=trace_cores,
    )
    return _gather(res.results), res


def kernel(features, W, b):
    (out0, out1), _ = _run({"features": features, "W": W, "b": b})
    return out0, out1


# revision 3
# speedup vs baseline: 1.9753x; 1.2184x over previous
"""Bass/Trainium2 kernel for nn_BranchedPolicyNetwork.

Computes out = tanh(features @ Wr + br) where
  features: [32768, 1024] f32
  W:        [64, 2, 1024] f32  (stacked per-branch Linear(L, 2) weights)
  b:        [64, 2] f32
returning (out[..., 0], out[..., 1]) as two [32768, 64] f32 arrays.

Strategy: data-parallel over batch across 8 NeuronCores (4096 rows each).
The TensorEngine contracts over the partition dim, so features are repacked
host-side into a transposed, tile-contiguous layout (free w.r.t. HW time).

The correctness gate is rel_l2 < 2e-2, which admits int8 quantization of
the feature stream: x is quantized host-side with per-feature absmax
scales (xq = rint(x/s_l), s_l folded into the fp16 weights, so no on-chip
rescale is needed; measured rel_l2 ~1.1e-2).  That halves HBM traffic vs
fp16 to ~5.5 MB/core.  The PE cannot consume int8, so each chunk is
upcast int8->fp16 on chip, split between the otherwise-idle DVE (6 of 8
ko slices, dual-port 2x mode ~1.85 elem/ns/partition) and ACT (2 of 8
slices via Copy activation) so neither cast engine exceeds the ~2.9 us
per-chunk DMA time.  GPSIMD must NOT be used: it shares SBUF ports with
the DVE and degrades concurrent DVE ops ~7x (measured).

Per-chunk steady state (1024 batch cols): DMA 2.9 us, DVE 3.4 us, ACT
2.9 us (casts + tanh), PE 3.5 us -- a four-engine ridge at the HBM
roofline for the quantized stream.
"""

import sys

for _p in ("/opt/trn_rl_repo", "/root/.axon_site"):
    if _p not in sys.path:
        sys.path.insert(0, _p)

import numpy as np

import concourse.mybir as mybir
import concourse.tile as tile
from concourse import bacc
from concourse.bass_utils import run_bass_kernel_spmd

# Problem shapes (hardcoded per contract)
B, L, A = 32768, 1024, 64
NCORES = 8
BS = B // NCORES          # 4096 batch rows per core
KO = L // 128             # 8 contraction slices
CH = 2 * A                # 128 output channels (c = k*64 + a)

F32 = mybir.dt.float32
F16 = mybir.dt.float16
I8 = mybir.dt.int8

# Chunk widths (batch columns per core).  Tapered tail: the final 256-col
# chunk leaves only ~1.5 us of cast+matmul+tanh+store after its last byte
# lands, instead of ~3 us for a 1024-col chunk.
CHUNKS = [1024, 1024, 1024, 768, 256]
assert sum(CHUNKS) == BS
CN_MAX = max(CHUNKS)
MM_N = 512        # moving free dim per matmul (one fp32 PSUM bank)
ACT_KO = 2        # ko slices cast by ACT; the rest go to DVE

_NC = None


def _build_nc():
    nc = bacc.Bacc()
    # x is packed chunk-major on the host: for each chunk (cn columns), the
    # per-partition bytes are one contiguous (ko, n) block of KO*cn int8s.
    xq = nc.dram_tensor("xq", [128, KO * BS], I8, kind="ExternalInput")
    wh = nc.dram_tensor("wh", [128, KO, CH], F16, kind="ExternalInput")
    bvec = nc.dram_tensor("bias", [CH, 1], F32, kind="ExternalInput")
    out = nc.dram_tensor("out", [CH, BS], F16, kind="ExternalOutput")

    with tile.TileContext(nc) as tc:
        with (
            tc.tile_pool(name="consts", bufs=1) as consts,
            tc.tile_pool(name="xqp", bufs=len(CHUNKS)) as xqp,
            tc.tile_pool(name="xfp", bufs=len(CHUNKS)) as xfp,
            tc.tile_pool(name="op", bufs=3) as op,
            tc.tile_pool(name="ps", bufs=3, space="PSUM") as ps,
            tc.tile_pool(name="warm", bufs=1, space="PSUM") as warm_ps,
        ):
            # PE warmup: ~10 dependency-free matmuls on zeroed tiles fill the
            # otherwise-idle window while the first loads stream in, so the
            # HAM clock gate is already at 8/8 (2.4 GHz) when real matmuls
            # start (saves the ~2x-slow cold ramp on the critical path).
            w_warm = consts.tile([128, CH], F16)
            nc.vector.memset(w_warm[:], 0.0)
            x_warm = consts.tile([128, MM_N], F16)
            nc.gpsimd.memset(x_warm[:], 0.0)
            pw = warm_ps.tile([CH, MM_N], F32)
            for i in range(10):
                nc.tensor.matmul(
                    pw[:], w_warm[:], x_warm[:], start=(i == 0), stop=(i == 9)
                )
            # Ring assignment: the Sync (SP) HWDGE ring is purely the x
            # stream in need-order.  The Scalar (ACT) ring loads the small
            # constants up front, then does casts + activations + out-stores.
            wh_sb = consts.tile([128, KO, CH], F16)
            nc.scalar.dma_start(wh_sb[:], wh[:])
            b_sb = consts.tile([CH, 1], F32)
            nc.scalar.dma_start(b_sb[:], bvec[:])

            # Issue ALL x loads up front on the Sync ring, split per ko-pair
            # so each chunk's casts/matmuls start as slices land.
            xq_tiles = []
            n0 = 0
            for ci, cn in enumerate(CHUNKS):
                off = KO * n0
                src = xq[:, off : off + KO * cn].rearrange(
                    "p (ko n) -> p ko n", ko=KO
                )
                xq_sb = xqp.tile([128, KO, CN_MAX], I8, tag="xq", name="xq_sb")[:, :, :cn]
                for k0 in range(0, KO, 2):
                    nc.sync.dma_start(
                        xq_sb[:, k0 : k0 + 2], src[:, k0 : k0 + 2]
                    )
                xq_tiles.append(xq_sb)
                n0 += cn

            # Per chunk: upcast int8 -> fp16 (ACT takes ko 0..ACT_KO-1, DVE
            # the rest in ko-pairs), matmul ko-major, tanh + store.
            # ACT emission order runs each chunk's cast one chunk ahead of
            # its tanh/store so casts never queue behind a stalled tanh.
            xf_tiles = []
            pts = []
            o_tiles = []
            for ci, cn in enumerate(CHUNKS):
                xq_sb = xq_tiles[ci]
                xf_sb = xfp.tile([128, KO, CN_MAX], F16, tag="xf", name="xf_sb")[:, :, :cn]
                # casts for this chunk
                nc.scalar.activation(
                    xf_sb[:, 0:ACT_KO],
                    xq_sb[:, 0:ACT_KO],
                    mybir.ActivationFunctionType.Copy,
                    scale=1.0,
                )
                for k0 in range(ACT_KO, KO, 2):
                    nc.vector.tensor_copy(
                        xf_sb[:, k0 : k0 + 2], xq_sb[:, k0 : k0 + 2]
                    )
                xf_tiles.append(xf_sb)
                # previous chunk's matmuls + tanh + store
                if ci > 0:
                    _emit_compute(nc, ps, op, wh_sb, b_sb, out, xf_tiles,
                                  pts, o_tiles, ci - 1)
            _emit_compute(nc, ps, op, wh_sb, b_sb, out, xf_tiles, pts,
                          o_tiles, len(CHUNKS) - 1)
    nc.compile()
    return nc


def _emit_compute(nc, ps, op, wh_sb, b_sb, out, xf_tiles, pts, o_tiles, ci):
    cn = CHUNKS[ci]
    n0 = sum(CHUNKS[:ci])
    xf_sb = xf_tiles[ci]
    pt = ps.tile([CH, CN_MAX], F32, tag="pt", name="pt")[:, :cn]
    for ko in range(KO):
        for s0 in range(0, cn, MM_N):
            s1 = min(s0 + MM_N, cn)
            nc.tensor.matmul(
                pt[:, s0:s1],
                wh_sb[:, ko],
                xf_sb[:, ko, s0:s1],
                start=(ko == 0),
                stop=(ko == KO - 1),
            )
    o_sb = op.tile([CH, CN_MAX], F16, tag="o", name="o_sb")[:, :cn]
    nc.scalar.activation(
        o_sb[:],
        pt[:],
        mybir.ActivationFunctionType.Tanh,
        bias=b_sb[:, 0:1],
        scale=1.0,
    )
    nc.scalar.dma_start(out[:, n0 : n0 + cn], o_sb[:])
    pts.append(pt)
    o_tiles.append(o_sb)


def _get_nc():
    global _NC
    if _NC is None:
        _NC = _build_nc()
    return _NC


def _pack_x(shard8):
    # shard8 [BS, L] int8 -> chunk-major [128, KO*BS]: per partition p,
    # chunk c occupies a contiguous (ko, n) block.
    shT = shard8.T  # [L, BS] view
    parts = []
    n0 = 0
    for cn in CHUNKS:
        blk = (
            shT[:, n0 : n0 + cn]
            .reshape(KO, 128, cn)
            .transpose(1, 0, 2)
            .reshape(128, KO * cn)
        )
        parts.append(blk)
        n0 += cn
    return np.ascontiguousarray(np.concatenate(parts, axis=1))


def _shard_inputs(features, W, b):
    features = np.ascontiguousarray(features, dtype=np.float32)
    W = np.ascontiguousarray(W, dtype=np.float32)
    b = np.ascontiguousarray(b, dtype=np.float32)

    # Per-feature absmax int8 quantization; scales fold into the weights.
    s = np.abs(features).max(axis=0) / 127.0  # [L]
    s = np.maximum(s, 1e-30)
    xq_all = np.rint(features / s[None, :]).astype(np.int8)  # [B, L]

    # Wr[l, c] with c = k*A + a; scale-folded fp16, device layout [p, ko, c]
    wr = W.transpose(2, 1, 0).reshape(L, CH)
    wr_h = (wr * s[:, None]).astype(np.float16)
    wh_dev = np.ascontiguousarray(wr_h.reshape(KO, 128, CH).transpose(1, 0, 2))
    b_dev = np.ascontiguousarray(b.transpose(1, 0).reshape(CH, 1))

    in_maps = []
    for i in range(NCORES):
        sh = xq_all[i * BS : (i + 1) * BS]  # [BS, L] int8
        in_maps.append(
            {
                "xq": _pack_x(sh),
                "wh": wh_dev,
                "bias": b_dev,
            }
        )
    return in_maps


def _gather(results):
    out0 = np.empty((B, A), dtype=np.float32)
    out1 = np.empty((B, A), dtype=np.float32)
    for i, r in enumerate(results):
        arr = r["out"].T.astype(np.float32)  # [CH, BS] -> [BS, CH]
        out0[i * BS : (i + 1) * BS] = arr[:, :A]
        out1[i * BS : (i + 1) * BS] = arr[:, A:]
    return out0, out1


def _run(inputs, trace=False, trace_cores=None):
    nc = _get_nc()
    in_maps = _shard_inputs(inputs["features"], inputs["W"], inputs["b"])
    res = run_bass_kernel_spmd(
        nc,
        in_maps,
        core_ids=list(range(NCORES)),
        trace=trace,
        trace_cores=trace_cores,
    )
    return _gather(res.results), res


def kernel(features, W, b):
    (out0, out1), _ = _run({"features": features, "W": W, "b": b})
    return out0, out1
